# revision 25
# baseline (speedup 1.0000x reference)
"""Trainium2 Bass kernel for nn_MultiHeadDotProductAttention_76725295776285.

Full multi-head attention (B=2, Q=K=4096, F=512, H=8, D=64) on 8 NeuronCores.

Sharding: core c handles batch b = c//4 and q-rows [(c%4)*1024, (c%4+1)*1024).
Each core computes all 8 heads for its q-slice (K/V projection for its batch is
recomputed on each of the 4 cores sharing that batch), so the output projection
sums over heads locally and no collective is needed.

Device-side dataflow (per core):
  - activations are fed pre-transposed ([F, seq]) so every matmul has its
    contraction dim on partitions with no on-chip transposes
  - Q^T [hd, q], K^T [hd, k] kept in float32r (TF32-like, full PE rate)
  - V evacuated to bf16 as [k, head, 65] with a ones column, so the AV matmul
    produces the softmax denominator in its 65th output row for free
  - S^T = K_h Q_h^T per head via two row-packed (tile_position) Kc=64 matmuls
  - softmax without max-subtraction (logits std ~8, |logit| < ~60 is fp32-safe)
  - exp on the Scalar engine PSUM->SBUF in [128, 1024] slabs, bf16 out
  - out^T accumulated in PSUM over all 32 k-chunks, normalized by the
    reciprocal denominator broadcast across partitions with a tiny matmul
  - output projection in bf16, accumulate over hd chunks, DMA out fp32
"""

import os
import sys

for _p in ("/opt/trn_rl_repo", "/root/.axon_site/_ro/trn_rl_repo"):
    if os.path.isdir(_p) and _p not in sys.path:
        sys.path.append(_p)

import numpy as np

import concourse.bacc as bacc
import concourse.tile as tile
from concourse import mybir
from concourse.bass_utils import run_bass_kernel_spmd

B, Q, K, F, H, D = 2, 4096, 4096, 512, 8, 64
HD = H * D            # 512
NCORES = 8
QSH = Q // 4          # 1024 q rows per core
FC = F // 128         # 4 F chunks
HDC = HD // 128       # 4 hd chunks
NKB = K // 512        # 8 k blocks (DMA/projection granularity)
NKC = K // 128        # 32 k chunks (attention granularity)
NQB = QSH // 512      # 2 q blocks per core
NHP = H // 2          # 4 head pairs

f32 = mybir.dt.float32
f32r = mybir.dt.float32r
f16 = mybir.dt.float16
bf16 = mybir.dt.bfloat16

_cache = {}
last_result = None  # BassKernelResults of the most recent run (for profiling)


def _build_program():
    nc = bacc.Bacc("TRN2", target_bir_lowering=False, debug=False,
                   num_devices=NCORES)

    xqT = nc.dram_tensor("xqT", [F, QSH], f16, kind="ExternalInput")
    xkvT = nc.dram_tensor("xkvT", [F, K], f16, kind="ExternalInput")
    wq = nc.dram_tensor("wq", [F, HD], f16, kind="ExternalInput")
    wk = nc.dram_tensor("wk", [F, HD], f16, kind="ExternalInput")
    wv = nc.dram_tensor("wv", [F, HD], f16, kind="ExternalInput")
    wo = nc.dram_tensor("wo", [HD, F], bf16, kind="ExternalInput")
    out = nc.dram_tensor("out", [QSH, F], f32, kind="ExternalOutput")

    # partition-major views: row index (c*128 + p) -> [p, c, :]
    xqT_r = xqT.rearrange("(c p) q -> p c q", p=128)
    xkvT_r = xkvT.rearrange("(c p) k -> p c k", p=128)
    wq_r = wq.rearrange("(c p) n -> p c n", p=128)
    wk_r = wk.rearrange("(c p) n -> p c n", p=128)
    wv_r = wv.rearrange("(c p) n -> p c n", p=128)
    wo_r = wo.rearrange("(c p) n -> p c n", p=128)

    with tile.TileContext(nc) as tc:
        with (
            tc.tile_pool(name="persist", bufs=1) as persist,
            tc.tile_pool(name="stream", bufs=3) as stream,
            tc.tile_pool(name="ptp", bufs=8) as ptp,
            tc.tile_pool(name="small", bufs=4) as small,
            tc.tile_pool(name="psum", bufs=2, space="PSUM") as psum,
            tc.tile_pool(name="dramp", bufs=4, space="DRAM") as dramp,
        ):
            # ---- persistent SBUF tensors ----
            qT_sb = persist.tile([128, HDC, QSH], f16, tag="qT")
            kT_sb = [persist.tile([128, HDC, 512], f16, tag=f"kT{kb}",
                                  name=f"kT{kb}")
                     for kb in range(NKB)]
            v_sb = [persist.tile([128, H, 65], bf16, tag=f"v{kc}",
                                 name=f"v{kc}")
                    for kc in range(NKC)]
            outT_sb = persist.tile([128, HDC, QSH], bf16, tag="outT")
            wk_sb = persist.tile([128, FC, HD], f16, tag="wk")
            wv_sb = persist.tile([128, FC, HD], f16, tag="wv")
            wq_sb = persist.tile([128, FC, HD], f16, tag="wq")
            wo_bf = persist.tile([128, HDC, F], bf16, tag="wo_bf")

            # ---- input DMAs + phase 1 (projections) ----
            # DMA order tuned so the kb0 attention chain starts ASAP.
            xq_blk = [None, None]
            xkv_blk = [None] * NKB

            xq_blk[0] = stream.tile([128, FC, 512], f16, tag="xblk",
                                    name="xqb0")
            xkv_blk[0] = stream.tile([128, FC, 512], f16, tag="xblk",
                                     name="xkvb0")
            for fc in range(FC):
                nc.sync.dma_start(out=wq_sb[:, fc, :], in_=wq_r[:, fc, :])
                nc.sync.dma_start(out=xq_blk[0][:, fc, :],
                                  in_=xqT_r[:, fc, 0:512])
            for fc in range(FC):
                nc.scalar.dma_start(out=wk_sb[:, fc, :], in_=wk_r[:, fc, :])
                nc.scalar.dma_start(out=xkv_blk[0][:, fc, :],
                                  in_=xkvT_r[:, fc, 0:512])
            nc.scalar.dma_start(out=wv_sb[:], in_=wv_r[:])
            xq_blk[1] = stream.tile([128, FC, 512], f16, tag="xblk",
                                    name="xqb1")
            nc.sync.dma_start(out=xq_blk[1][:], in_=xqT_r[:, :, 512:1024])
            nc.sync.dma_start(out=wo_bf[:], in_=wo_r[:])

            def emit_qproj(qb, hdcs=range(HDC)):
                for hdc in hdcs:
                    acc = psum.tile([128, 512], f32, tag="proj",
                                    name=f"qacc{qb}_{hdc}")
                    for fc in range(FC):
                        nc.tensor.matmul(
                            acc[:],
                            wq_sb[:, fc, hdc * 128:(hdc + 1) * 128],
                            xq_blk[qb][:, fc, :],
                            start=(fc == 0), stop=(fc == FC - 1),
                        )
                    nc.vector.tensor_copy(
                        qT_sb[:, hdc, qb * 512:(qb + 1) * 512], acc[:])

            def emit_kproj(kb, hdcs=range(HDC)):
                blk = xkv_blk[kb]
                for hdc in hdcs:
                    acc = psum.tile([128, 512], f32, tag="proj",
                                    name=f"kacc{kb}_{hdc}")
                    for fc in range(FC):
                        nc.tensor.matmul(
                            acc[:],
                            wk_sb[:, fc, hdc * 128:(hdc + 1) * 128],
                            blk[:, fc, :],
                            start=(fc == 0), stop=(fc == FC - 1),
                        )
                    nc.vector.tensor_copy(kT_sb[kb][:, hdc, :], acc[:])

            def emit_vproj(kb):
                blk = xkv_blk[kb]
                for ks in range(4):
                    kc = kb * 4 + ks
                    acc = psum.tile([128, 512], f32, tag="proj",
                                    name=f"vacc{kb}_{ks}")
                    for fc in range(FC):
                        nc.tensor.matmul(
                            acc[:],
                            blk[:, fc, ks * 128:(ks + 1) * 128],
                            wv_sb[:, fc, :],
                            start=(fc == 0), stop=(fc == FC - 1),
                        )
                    nc.vector.tensor_copy(
                        v_sb[kc][:, :, 0:64],
                        acc.rearrange("p (h d) -> p h d", h=H))
                    nc.vector.memset(v_sb[kc][:, :, 64:65], 1.0)

            # ---- attention emission helpers ----
            def emit_st_exp(hp, qb, kc, cached=False):
                kb, ks = kc // 4, kc % 4
                st = psum.tile([128, 2, 512], f32, tag="st",
                               name=f"st{hp}_{qb}_{kc}")
                for hi in range(2):
                    nc.tensor.matmul(
                        st[:, hi, :],
                        kT_sb[kb][hi * 64:(hi + 1) * 64, hp,
                                  ks * 128:(ks + 1) * 128],
                        qT_sb[hi * 64:(hi + 1) * 64, hp,
                              qb * 512:(qb + 1) * 512],
                        start=True, stop=True,
                        tile_position=(hi * 64, 0),
                    )
                pT = ptp.tile([128, 2, 512], bf16,
                              tag="pTc" if cached else "pT",
                              bufs=28 if cached else 11,
                              name=f"pT{hp}_{qb}_{kc}")
                nc.scalar.activation(
                    out=pT[:], in_=st[:],
                    func=mybir.ActivationFunctionType.Exp)
                return pT

            def emit_av(hp, qb, kc, av, pT):
                for hi in range(2):
                    nc.tensor.matmul(
                        av[hi][:],
                        v_sb[kc][:, hp * 2 + hi, :],
                        pT[:, hi, :],
                        start=(kc == 0), stop=(kc == NKC - 1),
                    )

            def alloc_av(hp, qb):
                return [psum.tile([65, 512], f32, tag="av",
                                  name=f"av{hp}_{qb}_{hi}")
                        for hi in range(2)]

            def emit_u_copies(hp, qb, av):
                us = []
                for hi in range(2):
                    u = small.tile([65, 512], f32r, tag="uav", bufs=3,
                                   name=f"u{hp}_{qb}_{hi}")
                    with nc.allow_low_precision(
                            reason="f32r attn output staging"):
                        nc.vector.tensor_copy(u[:], av[hi][:])
                    us.append(u)
                return (hp, qb, us)

            def emit_outproj(qb):
                # out[q, F] = sum_hd outT[hd, q]^T x wo[hd, F] for this qb
                for j in range(4):
                    qc = qb * 4 + j
                    acc = psum.tile([128, 512], f32, tag="proj",
                                    name=f"oacc{qc}")
                    for hdc in range(HDC):
                        nc.tensor.matmul(
                            acc[:],
                            outT_sb[:, hdc, qc * 128:(qc + 1) * 128],
                            wo_bf[:, hdc, :],
                            start=(hdc == 0), stop=(hdc == HDC - 1),
                        )
                    ostage = small.tile([128, 512], f32, tag="ost", bufs=2,
                                        name=f"ost{qc}")
                    nc.vector.tensor_copy(ostage[:], acc[:])
                    nc.sync.dma_start(out=out[qc * 128:(qc + 1) * 128, :],
                                      in_=ostage[:])

            def bcast_den(u, name):
                # replicate u's denominator row across 64 partitions: bounce
                # the row through DRAM, then read it back with a
                # partition-step-0 broadcast access pattern (GPSIMD ring,
                # off the critical DMA queue)
                import concourse.bass as _bass
                db = dramp.tile([1, 512], f32r, tag="dbc", name=f"d{name}")
                nc.gpsimd.dma_start(out=db[:], in_=u[64:65, :])
                bc = small.tile([64, 512], f32r, tag="bcd", bufs=2, name=name)
                src_ap = _bass.AP(tensor=db.tensor, offset=db.offset,
                                  ap=[[0, 64]] + [list(x) for x in db.ap[1:]])
                nc.gpsimd.dma_start(out=bc[:], in_=src_ap)
                return bc

            def emit_tail_rest(p):
                hp, qb, us = p
                bcs = [bcast_den(us[hi], f"bc{hp}_{qb}_{hi}")
                       for hi in range(2)]
                for hi in range(2):
                    rbc = small.tile([64, 512], f32, tag="rbc", bufs=2,
                                     name=f"rbc{hp}_{qb}_{hi}")
                    nc.vector.reciprocal(rbc[:], bcs[hi][:])
                    nc.vector.tensor_mul(
                        outT_sb[hi * 64:(hi + 1) * 64, hp,
                                qb * 512:(qb + 1) * 512],
                        us[hi][0:64, :], rbc[:])

            # ---- phase 2: two-cursor S^T/exp vs AV emission ----
            # The S^T+exp (SE) stream runs up to CAP slabs ahead of the AV
            # stream, decoupled through the pT pool. During the ramp the SE
            # order is kb-major across the first two iterations so the
            # Scalar engine is fed ~8 exps per projected k-block; afterwards
            # it is iteration-major. The AV stream is iteration-major
            # (PSUM accumulator lifetime), with the normalization tails and
            # per-qb output projections woven in at fixed offsets.
            ROLL = 11          # rolling slab pool depth (tag pT)
            iters = [(hp, qb) for hp in range(NHP) for qb in range(NQB)]
            NIT = len(iters)

            # SE order: ramp kb-major — iteration 0 always, iteration 1
            # cached (long-lived slabs) for kb>=4 — then iteration 1's
            # remaining k-chunks, then iterations 2.. in order.
            se_list = []   # (iter_idx, kc, cached)
            for kb in range(NKB):
                se_list.extend((0, kb * 4 + ks, False) for ks in range(4))
                if kb <= 6:
                    se_list.extend((1, kb * 4 + ks, True) for ks in range(4))
            se_list.extend((1, kc, False) for kc in range(28, 32))
            for i in range(2, NIT):
                se_list.extend((i, kc, False) for kc in range(NKC))
            av_list = [(i, kc) for i in range(NIT) for kc in range(NKC)]
            av_idx = {step: j for j, step in enumerate(av_list)}
            # rolling-tag allocation order, for the FIFO slot-reuse guard
            roll_steps = [s for s in se_list if not s[2]]

            cache = {}
            state = dict(se_pos=0, roll_pos=0, av_pos=0, pending=None,
                         outproj_due=None, av_tiles=None, kb_done=0)

            def pump_se(budget):
                n = 0
                while n < budget and state["se_pos"] < len(se_list):
                    i, kc, cached = se_list[state["se_pos"]]
                    if kc // 4 >= state["kb_done"]:
                        break  # K^T/V for this k-block not yet emitted
                    if not cached:
                        r = state["roll_pos"]
                        if r >= ROLL:
                            blocker = roll_steps[r - ROLL]
                            if av_idx[blocker[:2]] >= state["av_pos"]:
                                break  # rolling slot's consumer not emitted
                        state["roll_pos"] = r + 1
                    hp, qb = iters[i]
                    cache[(i, kc)] = emit_st_exp(hp, qb, kc, cached)
                    state["se_pos"] += 1
                    n += 1

            def emit_av_step():
                i, kc = av_list[state["av_pos"]]
                hp, qb = iters[i]
                if kc == 0:
                    state["av_tiles"] = alloc_av(hp, qb)
                emit_av(hp, qb, kc, state["av_tiles"], cache.pop((i, kc)))
                if kc == 4 and state["pending"] is not None:
                    emit_tail_rest(state["pending"])
                    state["outproj_due"] = (state["pending"][1]
                                            if state["pending"][0] == NHP - 1
                                            else None)
                    state["pending"] = None
                if kc == 16 and state["outproj_due"] is not None:
                    emit_outproj(state["outproj_due"])
                    state["outproj_due"] = None
                if kc == NKC - 1 and i < NIT - 1:
                    state["pending"] = emit_u_copies(hp, qb, state["av_tiles"])
                state["av_pos"] += 1

            def av_ready():
                # next AV step's slab is already emitted
                return (state["av_pos"] < len(av_list)
                        and av_list[state["av_pos"]] in cache)

            emit_qproj(0, hdcs=[0])
            for kb in range(NKB):
                if kb > 0:
                    xkv_blk[kb] = stream.tile([128, FC, 512], f16,
                                              tag="xblk", name=f"xkvb{kb}")
                    nc.sync.dma_start(
                        out=xkv_blk[kb][:],
                        in_=xkvT_r[:, :, kb * 512:(kb + 1) * 512])
                # hdc0 of K^T unblocks all head-pair-0 S^T for this k-block
                emit_kproj(kb, hdcs=[0])
                state["kb_done"] = kb + 1
                pump_se(4)
                if kb == 0:
                    emit_qproj(1, hdcs=[0])
                pump_se(4)
                emit_kproj(kb, hdcs=[1, 2, 3])
                if kb == 0:
                    emit_qproj(0, hdcs=[1, 2, 3])
                pump_se(2)
                emit_vproj(kb)
                if kb == 0:
                    emit_qproj(1, hdcs=[1, 2, 3])
                pump_se(8)
                while av_ready():
                    emit_av_step()
                    pump_se(1)

            while state["av_pos"] < len(av_list):
                pump_se(2)
                if not av_ready():
                    pump_se(len(se_list))
                emit_av_step()

            # final tail, pipelined per 128-wide q chunk so the output
            # projection starts as soon as each slice is normalized
            fi = NIT - 1
            fhp, fqb = iters[fi]
            fus = emit_u_copies(fhp, fqb, state["av_tiles"])[2]
            fbcs = [bcast_den(fus[hi], f"fbc{hi}") for hi in range(2)]
            for j in range(4):
                qc = fqb * 4 + j
                js = slice(j * 128, (j + 1) * 128)
                for hi in range(2):
                    rbc = small.tile([64, 128], f32, tag="frbc", bufs=2,
                                     name=f"frbc{j}_{hi}")
                    nc.vector.reciprocal(rbc[:], fbcs[hi][:, js])
                    nc.vector.tensor_mul(
                        outT_sb[hi * 64:(hi + 1) * 64, fhp,
                                fqb * 512 + j * 128:fqb * 512 + (j + 1) * 128],
                        fus[hi][0:64, js], rbc[:])
                acc = psum.tile([128, 512], f32, tag="st", name=f"foacc{qc}")
                for hdc in range(HDC):
                    nc.tensor.matmul(
                        acc[:],
                        outT_sb[:, hdc, qc * 128:(qc + 1) * 128],
                        wo_bf[:, hdc, :],
                        start=(hdc == 0), stop=(hdc == HDC - 1),
                    )
                ostage = small.tile([128, 512], f32, tag="ost", bufs=2,
                                    name=f"fost{qc}")
                nc.vector.tensor_copy(ostage[:], acc[:])
                nc.sync.dma_start(out=out[qc * 128:(qc + 1) * 128, :],
                                  in_=ostage[:])

    nc.compile()
    return nc


def kernel(**inputs):
    global last_result
    import ml_dtypes
    inputs_q = np.asarray(inputs["inputs_q"], dtype=np.float32)
    inputs_kv = np.asarray(inputs["inputs_kv"], dtype=np.float32)
    Wq = np.asarray(inputs["Wq"], dtype=np.float32).reshape(F, HD).astype(np.float16)
    Wk = np.asarray(inputs["Wk"], dtype=np.float32).reshape(F, HD).astype(np.float16)
    Wv = np.asarray(inputs["Wv"], dtype=np.float32).reshape(F, HD).astype(np.float16)
    Wo = np.asarray(inputs["Wo"], dtype=np.float32).reshape(HD, F).astype(ml_dtypes.bfloat16)

    if "nc" not in _cache:
        _cache["nc"] = _build_program()
    nc = _cache["nc"]

    xkvT = [np.ascontiguousarray(inputs_kv[b].T).astype(np.float16) for b in range(B)]
    in_maps = []
    for c in range(NCORES):
        b, qi = c // 4, c % 4
        in_maps.append({
            "xqT": np.ascontiguousarray(
                inputs_q[b, qi * QSH:(qi + 1) * QSH, :].T).astype(np.float16),
            "xkvT": xkvT[b],
            "wq": Wq, "wk": Wk, "wv": Wv, "wo": Wo,
        })

    res = run_bass_kernel_spmd(nc, in_maps, core_ids=list(range(NCORES)))
    last_result = res

    out = np.empty((B, Q, F), dtype=np.float32)
    for c in range(NCORES):
        b, qi = c // 4, c % 4
        out[b, qi * QSH:(qi + 1) * QSH, :] = res.results[c]["out"]
    return out


# revision 27
# speedup vs baseline: 1.0588x; 1.0588x over previous
"""Trainium2 Bass kernel for nn_MultiHeadDotProductAttention_76725295776285.

Full multi-head attention (B=2, Q=K=4096, F=512, H=8, D=64) on 8 NeuronCores.

Sharding: core c handles batch b = c//4 and q-rows [(c%4)*1024, (c%4+1)*1024).
Each core computes all 8 heads for its q-slice (K/V projection for its batch is
recomputed on each of the 4 cores sharing that batch), so the output projection
sums over heads locally and no collective is needed.

Device-side dataflow (per core):
  - activations are fed pre-transposed ([F, seq]) so every matmul has its
    contraction dim on partitions with no on-chip transposes
  - Q^T [hd, q], K^T [hd, k] kept in float32r (TF32-like, full PE rate)
  - V evacuated to bf16 as [k, head, 65] with a ones column, so the AV matmul
    produces the softmax denominator in its 65th output row for free
  - S^T = K_h Q_h^T per head via two row-packed (tile_position) Kc=64 matmuls
  - softmax without max-subtraction (logits std ~8, |logit| < ~60 is fp32-safe)
  - exp on the Scalar engine PSUM->SBUF in [128, 1024] slabs, bf16 out
  - out^T accumulated in PSUM over all 32 k-chunks, normalized by the
    reciprocal denominator broadcast across partitions with a tiny matmul
  - output projection in bf16, accumulate over hd chunks, DMA out fp32
"""

import os
import sys

for _p in ("/opt/trn_rl_repo", "/root/.axon_site/_ro/trn_rl_repo"):
    if os.path.isdir(_p) and _p not in sys.path:
        sys.path.append(_p)

import numpy as np

import concourse.bacc as bacc
import concourse.tile as tile
from concourse import mybir
from concourse.bass_utils import run_bass_kernel_spmd

B, Q, K, F, H, D = 2, 4096, 4096, 512, 8, 64
HD = H * D            # 512
NCORES = 8
QSH = Q // 4          # 1024 q rows per core
FC = F // 128         # 4 F chunks
HDC = HD // 128       # 4 hd chunks
NKB = K // 512        # 8 k blocks (DMA/projection granularity)
NKC = K // 128        # 32 k chunks (attention granularity)
NQB = QSH // 512      # 2 q blocks per core
NHP = H // 2          # 4 head pairs

f32 = mybir.dt.float32
f32r = mybir.dt.float32r
f16 = mybir.dt.float16
bf16 = mybir.dt.bfloat16

_cache = {}
last_result = None  # BassKernelResults of the most recent run (for profiling)


def _build_program():
    nc = bacc.Bacc("TRN2", target_bir_lowering=False, debug=False,
                   num_devices=NCORES)

    xqT = nc.dram_tensor("xqT", [F, QSH], f16, kind="ExternalInput")
    xkvT = nc.dram_tensor("xkvT", [F, K], f16, kind="ExternalInput")
    wq = nc.dram_tensor("wq", [F, HD], f16, kind="ExternalInput")
    wk = nc.dram_tensor("wk", [F, HD], f16, kind="ExternalInput")
    wv = nc.dram_tensor("wv", [F, HD], f16, kind="ExternalInput")
    wo = nc.dram_tensor("wo", [HD, F], bf16, kind="ExternalInput")
    ones64 = nc.dram_tensor("ones64", [1, 64], f32r, kind="ExternalInput")
    out = nc.dram_tensor("out", [QSH, F], f32, kind="ExternalOutput")

    # partition-major views: row index (c*128 + p) -> [p, c, :]
    xqT_r = xqT.rearrange("(c p) q -> p c q", p=128)
    xkvT_r = xkvT.rearrange("(c p) k -> p c k", p=128)
    wq_r = wq.rearrange("(c p) n -> p c n", p=128)
    wk_r = wk.rearrange("(c p) n -> p c n", p=128)
    wv_r = wv.rearrange("(c p) n -> p c n", p=128)
    wo_r = wo.rearrange("(c p) n -> p c n", p=128)

    with tile.TileContext(nc) as tc:
        with (
            tc.tile_pool(name="persist", bufs=1) as persist,
            tc.tile_pool(name="stream", bufs=3) as stream,
            tc.tile_pool(name="ptp", bufs=8) as ptp,
            tc.tile_pool(name="small", bufs=4) as small,
            tc.tile_pool(name="psum", bufs=2, space="PSUM") as psum,
        ):
            # ---- persistent SBUF tensors ----
            qT_sb = persist.tile([128, HDC, QSH], f16, tag="qT")
            kT_sb = [persist.tile([128, HDC, 512], f16, tag=f"kT{kb}",
                                  name=f"kT{kb}")
                     for kb in range(NKB)]
            v_sb = [persist.tile([128, H, 65], bf16, tag=f"v{kc}",
                                 name=f"v{kc}")
                    for kc in range(NKC)]
            outT_sb = persist.tile([128, HDC, QSH], bf16, tag="outT")
            wk_sb = persist.tile([128, FC, HD], f16, tag="wk")
            wv_sb = persist.tile([128, FC, HD], f16, tag="wv")
            wq_sb = persist.tile([128, FC, HD], f16, tag="wq")
            wo_bf = persist.tile([128, HDC, F], bf16, tag="wo_bf")
            ones_sb = persist.tile([65, 64], f32r, tag="ones")

            # ---- input DMAs + phase 1 (projections) ----
            # DMA order tuned so the kb0 attention chain starts ASAP.
            xq_blk = [None, None]
            xkv_blk = [None] * NKB

            xq_blk[0] = stream.tile([128, FC, 512], f16, tag="xblk",
                                    name="xqb0")
            xkv_blk[0] = stream.tile([128, FC, 512], f16, tag="xblk",
                                     name="xkvb0")
            for fc in range(FC):
                nc.sync.dma_start(out=wq_sb[:, fc, :], in_=wq_r[:, fc, :])
                nc.sync.dma_start(out=xq_blk[0][:, fc, :],
                                  in_=xqT_r[:, fc, 0:512])
            for fc in range(FC):
                nc.scalar.dma_start(out=wk_sb[:, fc, :], in_=wk_r[:, fc, :])
                nc.scalar.dma_start(out=xkv_blk[0][:, fc, :],
                                  in_=xkvT_r[:, fc, 0:512])
            nc.scalar.dma_start(out=wv_sb[:], in_=wv_r[:])
            xq_blk[1] = stream.tile([128, FC, 512], f16, tag="xblk",
                                    name="xqb1")
            nc.sync.dma_start(out=xq_blk[1][:], in_=xqT_r[:, :, 512:1024])
            nc.sync.dma_start(out=wo_bf[:], in_=wo_r[:])
            nc.sync.dma_start(out=ones_sb[64:65, :], in_=ones64[:])

            def emit_qproj(qb, hdcs=range(HDC)):
                for hdc in hdcs:
                    acc = psum.tile([128, 512], f32, tag="proj",
                                    name=f"qacc{qb}_{hdc}")
                    for fc in range(FC):
                        nc.tensor.matmul(
                            acc[:],
                            wq_sb[:, fc, hdc * 128:(hdc + 1) * 128],
                            xq_blk[qb][:, fc, :],
                            start=(fc == 0), stop=(fc == FC - 1),
                        )
                    nc.vector.tensor_copy(
                        qT_sb[:, hdc, qb * 512:(qb + 1) * 512], acc[:])

            def emit_kproj(kb, hdcs=range(HDC)):
                blk = xkv_blk[kb]
                for hdc in hdcs:
                    acc = psum.tile([128, 512], f32, tag="proj",
                                    name=f"kacc{kb}_{hdc}")
                    for fc in range(FC):
                        nc.tensor.matmul(
                            acc[:],
                            wk_sb[:, fc, hdc * 128:(hdc + 1) * 128],
                            blk[:, fc, :],
                            start=(fc == 0), stop=(fc == FC - 1),
                        )
                    nc.vector.tensor_copy(kT_sb[kb][:, hdc, :], acc[:])

            def emit_vproj(kb):
                blk = xkv_blk[kb]
                for ks in range(4):
                    kc = kb * 4 + ks
                    acc = psum.tile([128, 512], f32, tag="proj",
                                    name=f"vacc{kb}_{ks}")
                    for fc in range(FC):
                        nc.tensor.matmul(
                            acc[:],
                            blk[:, fc, ks * 128:(ks + 1) * 128],
                            wv_sb[:, fc, :],
                            start=(fc == 0), stop=(fc == FC - 1),
                        )
                    nc.vector.tensor_copy(
                        v_sb[kc][:, :, 0:64],
                        acc.rearrange("p (h d) -> p h d", h=H))
                    nc.vector.memset(v_sb[kc][:, :, 64:65], 1.0)

            # ---- attention emission helpers ----
            def emit_st_exp(hp, qb, kc, cached=False):
                kb, ks = kc // 4, kc % 4
                st = psum.tile([128, 2, 512], f32, tag="st",
                               name=f"st{hp}_{qb}_{kc}")
                for hi in range(2):
                    nc.tensor.matmul(
                        st[:, hi, :],
                        kT_sb[kb][hi * 64:(hi + 1) * 64, hp,
                                  ks * 128:(ks + 1) * 128],
                        qT_sb[hi * 64:(hi + 1) * 64, hp,
                              qb * 512:(qb + 1) * 512],
                        start=True, stop=True,
                        tile_position=(hi * 64, 0),
                    )
                pT = ptp.tile([128, 2, 512], bf16,
                              tag="pTc" if cached else "pT",
                              bufs=28 if cached else 12,
                              name=f"pT{hp}_{qb}_{kc}")
                nc.scalar.activation(
                    out=pT[:], in_=st[:],
                    func=mybir.ActivationFunctionType.Exp)
                return pT

            def emit_av(hp, qb, kc, av, pT):
                for hi in range(2):
                    nc.tensor.matmul(
                        av[hi][:],
                        v_sb[kc][:, hp * 2 + hi, :],
                        pT[:, hi, :],
                        start=(kc == 0), stop=(kc == NKC - 1),
                    )

            def alloc_av(hp, qb):
                return [psum.tile([65, 512], f32, tag="av",
                                  name=f"av{hp}_{qb}_{hi}")
                        for hi in range(2)]

            def emit_u_copies(hp, qb, av):
                us = []
                for hi in range(2):
                    u = small.tile([65, 512], f32r, tag="uav", bufs=3,
                                   name=f"u{hp}_{qb}_{hi}")
                    with nc.allow_low_precision(
                            reason="f32r attn output staging"):
                        nc.vector.tensor_copy(u[:], av[hi][:])
                    us.append(u)
                return (hp, qb, us)

            def emit_outproj(qb):
                # out[q, F] = sum_hd outT[hd, q]^T x wo[hd, F] for this qb
                for j in range(4):
                    qc = qb * 4 + j
                    acc = psum.tile([128, 512], f32, tag="proj",
                                    name=f"oacc{qc}")
                    for hdc in range(HDC):
                        nc.tensor.matmul(
                            acc[:],
                            outT_sb[:, hdc, qc * 128:(qc + 1) * 128],
                            wo_bf[:, hdc, :],
                            start=(hdc == 0), stop=(hdc == HDC - 1),
                        )
                    ostage = small.tile([128, 512], f32, tag="ost", bufs=2,
                                        name=f"ost{qc}")
                    nc.vector.tensor_copy(ostage[:], acc[:])
                    nc.sync.dma_start(out=out[qc * 128:(qc + 1) * 128, :],
                                      in_=ostage[:])

            def bcast_den(u, name):
                # broadcast u's denominator row across 64 partitions via a
                # tiny ones^T matmul (PE waits only on the fast av->u copy)
                bc = psum.tile([64, 512], f32, tag="proj", name=name)
                nc.tensor.matmul(bc[:], ones_sb[64:65, :], u[64:65, :],
                                 start=True, stop=True)
                return bc

            def emit_tail_rest(p):
                hp, qb, us = p
                bcs = [bcast_den(us[hi], f"bc{hp}_{qb}_{hi}")
                       for hi in range(2)]
                for hi in range(2):
                    rbc = small.tile([64, 512], f32, tag="rbc", bufs=2,
                                     name=f"rbc{hp}_{qb}_{hi}")
                    nc.vector.reciprocal(rbc[:], bcs[hi][:])
                    nc.vector.tensor_mul(
                        outT_sb[hi * 64:(hi + 1) * 64, hp,
                                qb * 512:(qb + 1) * 512],
                        us[hi][0:64, :], rbc[:])

            # ---- phase 2: two-cursor S^T/exp vs AV emission ----
            # The S^T+exp (SE) stream runs up to CAP slabs ahead of the AV
            # stream, decoupled through the pT pool. During the ramp the SE
            # order is kb-major across the first two iterations so the
            # Scalar engine is fed ~8 exps per projected k-block; afterwards
            # it is iteration-major. The AV stream is iteration-major
            # (PSUM accumulator lifetime), with the normalization tails and
            # per-qb output projections woven in at fixed offsets.
            ROLL = 12          # rolling slab pool depth (tag pT)
            iters = [(hp, qb) for hp in range(NHP) for qb in range(NQB)]
            NIT = len(iters)

            # SE order: ramp kb-major — iteration 0 always, iteration 1
            # cached (long-lived slabs) for kb>=4 — then iteration 1's
            # remaining k-chunks, then iterations 2.. in order.
            se_list = []   # (iter_idx, kc, cached)
            for kb in range(NKB):
                se_list.extend((0, kb * 4 + ks, False) for ks in range(4))
                if kb <= 6:
                    se_list.extend((1, kb * 4 + ks, True) for ks in range(4))
            se_list.extend((1, kc, False) for kc in range(28, 32))
            for i in range(2, NIT):
                se_list.extend((i, kc, False) for kc in range(NKC))
            av_list = [(i, kc) for i in range(NIT) for kc in range(NKC)]
            av_idx = {step: j for j, step in enumerate(av_list)}
            # rolling-tag allocation order, for the FIFO slot-reuse guard
            roll_steps = [s for s in se_list if not s[2]]

            cache = {}
            state = dict(se_pos=0, roll_pos=0, av_pos=0, pending=None,
                         outproj_due=None, av_tiles=None, kb_done=0)

            def pump_se(budget):
                n = 0
                while n < budget and state["se_pos"] < len(se_list):
                    i, kc, cached = se_list[state["se_pos"]]
                    if kc // 4 >= state["kb_done"]:
                        break  # K^T/V for this k-block not yet emitted
                    if not cached:
                        r = state["roll_pos"]
                        if r >= ROLL:
                            blocker = roll_steps[r - ROLL]
                            if av_idx[blocker[:2]] >= state["av_pos"]:
                                break  # rolling slot's consumer not emitted
                        state["roll_pos"] = r + 1
                    hp, qb = iters[i]
                    cache[(i, kc)] = emit_st_exp(hp, qb, kc, cached)
                    state["se_pos"] += 1
                    n += 1

            def emit_av_step():
                i, kc = av_list[state["av_pos"]]
                hp, qb = iters[i]
                if kc == 0:
                    state["av_tiles"] = alloc_av(hp, qb)
                emit_av(hp, qb, kc, state["av_tiles"], cache.pop((i, kc)))
                if kc == 4 and state["pending"] is not None:
                    emit_tail_rest(state["pending"])
                    state["outproj_due"] = (state["pending"][1]
                                            if state["pending"][0] == NHP - 1
                                            else None)
                    state["pending"] = None
                if kc == 16 and state["outproj_due"] is not None:
                    emit_outproj(state["outproj_due"])
                    state["outproj_due"] = None
                if kc == NKC - 1 and i < NIT - 1:
                    state["pending"] = emit_u_copies(hp, qb, state["av_tiles"])
                state["av_pos"] += 1

            def av_ready():
                # next AV step's slab is already emitted
                return (state["av_pos"] < len(av_list)
                        and av_list[state["av_pos"]] in cache)

            emit_qproj(0, hdcs=[0])
            for kb in range(NKB):
                if kb > 0:
                    xkv_blk[kb] = stream.tile([128, FC, 512], f16,
                                              tag="xblk", name=f"xkvb{kb}")
                    nc.sync.dma_start(
                        out=xkv_blk[kb][:],
                        in_=xkvT_r[:, :, kb * 512:(kb + 1) * 512])
                # hdc0 of K^T unblocks all head-pair-0 S^T for this k-block
                emit_kproj(kb, hdcs=[0])
                state["kb_done"] = kb + 1
                pump_se(4)
                if kb == 0:
                    emit_qproj(1, hdcs=[0])
                pump_se(4)
                emit_kproj(kb, hdcs=[1, 2, 3])
                if kb == 0:
                    emit_qproj(0, hdcs=[1, 2, 3])
                pump_se(2)
                emit_vproj(kb)
                if kb == 0:
                    emit_qproj(1, hdcs=[1, 2, 3])
                pump_se(8)
                while av_ready():
                    emit_av_step()
                    pump_se(1)

            while state["av_pos"] < len(av_list):
                pump_se(2)
                if not av_ready():
                    pump_se(len(se_list))
                emit_av_step()

            # final tail, pipelined per 128-wide q chunk so the output
            # projection starts as soon as each slice is normalized
            fi = NIT - 1
            fhp, fqb = iters[fi]
            fus = emit_u_copies(fhp, fqb, state["av_tiles"])[2]
            fbcs = [bcast_den(fus[hi], f"fbc{hi}") for hi in range(2)]
            for j in range(4):
                qc = fqb * 4 + j
                js = slice(j * 128, (j + 1) * 128)
                for hi in range(2):
                    rbc = small.tile([64, 128], f32, tag="frbc",
                                     name=f"frbc{j}_{hi}")
                    nc.vector.reciprocal(rbc[:], fbcs[hi][:, js])
                    nc.vector.tensor_mul(
                        outT_sb[hi * 64:(hi + 1) * 64, fhp,
                                fqb * 512 + j * 128:fqb * 512 + (j + 1) * 128],
                        fus[hi][0:64, js], rbc[:])
                acc = psum.tile([128, 512], f32, tag="st", name=f"foacc{qc}")
                for hdc in range(HDC):
                    nc.tensor.matmul(
                        acc[:],
                        outT_sb[:, hdc, qc * 128:(qc + 1) * 128],
                        wo_bf[:, hdc, :],
                        start=(hdc == 0), stop=(hdc == HDC - 1),
                    )
                ostage = small.tile([128, 512], f32, tag="ost", bufs=2,
                                    name=f"fost{qc}")
                nc.vector.tensor_copy(ostage[:], acc[:])
                nc.sync.dma_start(out=out[qc * 128:(qc + 1) * 128, :],
                                  in_=ostage[:])

    nc.compile()
    return nc


def kernel(**inputs):
    global last_result
    import ml_dtypes
    inputs_q = np.asarray(inputs["inputs_q"], dtype=np.float32)
    inputs_kv = np.asarray(inputs["inputs_kv"], dtype=np.float32)
    Wq = np.asarray(inputs["Wq"], dtype=np.float32).reshape(F, HD).astype(np.float16)
    Wk = np.asarray(inputs["Wk"], dtype=np.float32).reshape(F, HD).astype(np.float16)
    Wv = np.asarray(inputs["Wv"], dtype=np.float32).reshape(F, HD).astype(np.float16)
    Wo = np.asarray(inputs["Wo"], dtype=np.float32).reshape(HD, F).astype(ml_dtypes.bfloat16)
    ones = np.ones((1, 64), dtype=np.float32)

    if "nc" not in _cache:
        _cache["nc"] = _build_program()
    nc = _cache["nc"]

    xkvT = [np.ascontiguousarray(inputs_kv[b].T).astype(np.float16) for b in range(B)]
    in_maps = []
    for c in range(NCORES):
        b, qi = c // 4, c % 4
        in_maps.append({
            "xqT": np.ascontiguousarray(
                inputs_q[b, qi * QSH:(qi + 1) * QSH, :].T).astype(np.float16),
            "xkvT": xkvT[b],
            "wq": Wq, "wk": Wk, "wv": Wv, "wo": Wo,
            "ones64": ones,
        })

    res = run_bass_kernel_spmd(nc, in_maps, core_ids=list(range(NCORES)))
    last_result = res

    out = np.empty((B, Q, F), dtype=np.float32)
    for c in range(NCORES):
        b, qi = c // 4, c % 4
        out[b, qi * QSH:(qi + 1) * QSH, :] = res.results[c]["out"]
    return out


# revision 28
# speedup vs baseline: 1.0703x; 1.0109x over previous
"""Trainium2 Bass kernel for nn_MultiHeadDotProductAttention_76725295776285.

Full multi-head attention (B=2, Q=K=4096, F=512, H=8, D=64) on 8 NeuronCores.

Sharding: core c handles batch b = c//4 and q-rows [(c%4)*1024, (c%4+1)*1024).
Each core computes all 8 heads for its q-slice (K/V projection for its batch is
recomputed on each of the 4 cores sharing that batch), so the output projection
sums over heads locally and no collective is needed.

Device-side dataflow (per core):
  - activations are fed pre-transposed ([F, seq]) so every matmul has its
    contraction dim on partitions with no on-chip transposes
  - Q^T [hd, q], K^T [hd, k] kept in float32r (TF32-like, full PE rate)
  - V evacuated to bf16 as [k, head, 65] with a ones column, so the AV matmul
    produces the softmax denominator in its 65th output row for free
  - S^T = K_h Q_h^T per head via two row-packed (tile_position) Kc=64 matmuls
  - softmax without max-subtraction (logits std ~8, |logit| < ~60 is fp32-safe)
  - exp on the Scalar engine PSUM->SBUF in [128, 1024] slabs, bf16 out
  - out^T accumulated in PSUM over all 32 k-chunks, normalized by the
    reciprocal denominator broadcast across partitions with a tiny matmul
  - output projection in bf16, accumulate over hd chunks, DMA out fp32
"""

import os
import sys

for _p in ("/opt/trn_rl_repo", "/root/.axon_site/_ro/trn_rl_repo"):
    if os.path.isdir(_p) and _p not in sys.path:
        sys.path.append(_p)

import numpy as np

import concourse.bacc as bacc
import concourse.tile as tile
from concourse import mybir
from concourse.bass_utils import run_bass_kernel_spmd

B, Q, K, F, H, D = 2, 4096, 4096, 512, 8, 64
HD = H * D            # 512
NCORES = 8
QSH = Q // 4          # 1024 q rows per core
FC = F // 128         # 4 F chunks
HDC = HD // 128       # 4 hd chunks
NKB = K // 512        # 8 k blocks (DMA/projection granularity)
NKC = K // 128        # 32 k chunks (attention granularity)
NQB = QSH // 512      # 2 q blocks per core
NHP = H // 2          # 4 head pairs

f32 = mybir.dt.float32
f32r = mybir.dt.float32r
f16 = mybir.dt.float16
bf16 = mybir.dt.bfloat16

_cache = {}
last_result = None  # BassKernelResults of the most recent run (for profiling)


def _build_program():
    nc = bacc.Bacc("TRN2", target_bir_lowering=False, debug=False,
                   num_devices=NCORES)

    xqT = nc.dram_tensor("xqT", [F, QSH], f16, kind="ExternalInput")
    xkvT = nc.dram_tensor("xkvT", [F, K], f16, kind="ExternalInput")
    wq = nc.dram_tensor("wq", [F, HD], f16, kind="ExternalInput")
    wk = nc.dram_tensor("wk", [F, HD], f16, kind="ExternalInput")
    wv = nc.dram_tensor("wv", [F, HD], f16, kind="ExternalInput")
    wo = nc.dram_tensor("wo", [HD, F], bf16, kind="ExternalInput")
    ones64 = nc.dram_tensor("ones64", [1, 64], f32r, kind="ExternalInput")
    out = nc.dram_tensor("out", [QSH, F], f32, kind="ExternalOutput")

    # partition-major views: row index (c*128 + p) -> [p, c, :]
    xqT_r = xqT.rearrange("(c p) q -> p c q", p=128)
    xkvT_r = xkvT.rearrange("(c p) k -> p c k", p=128)
    wq_r = wq.rearrange("(c p) n -> p c n", p=128)
    wk_r = wk.rearrange("(c p) n -> p c n", p=128)
    wv_r = wv.rearrange("(c p) n -> p c n", p=128)
    wo_r = wo.rearrange("(c p) n -> p c n", p=128)

    with tile.TileContext(nc) as tc:
        with (
            tc.tile_pool(name="persist", bufs=1) as persist,
            tc.tile_pool(name="stream", bufs=3) as stream,
            tc.tile_pool(name="ptp", bufs=8) as ptp,
            tc.tile_pool(name="small", bufs=4) as small,
            tc.tile_pool(name="psum", bufs=2, space="PSUM") as psum,
        ):
            # ---- persistent SBUF tensors ----
            qT_sb = persist.tile([128, HDC, QSH], f16, tag="qT")
            kT_sb = [persist.tile([128, HDC, 512], f16, tag=f"kT{kb}",
                                  name=f"kT{kb}")
                     for kb in range(NKB)]
            v_sb = [persist.tile([128, H, 65], bf16, tag=f"v{kc}",
                                 name=f"v{kc}")
                    for kc in range(NKC)]
            outT_sb = persist.tile([128, HDC, QSH], bf16, tag="outT")
            wk_sb = persist.tile([128, FC, HD], f16, tag="wk")
            wv_sb = persist.tile([128, FC, HD], f16, tag="wv")
            wq_sb = persist.tile([128, FC, HD], f16, tag="wq")
            wo_bf = persist.tile([128, HDC, F], bf16, tag="wo_bf")
            ones_sb = persist.tile([65, 64], f32r, tag="ones")

            # ---- input DMAs + phase 1 (projections) ----
            # DMA order tuned so the kb0 attention chain starts ASAP.
            xq_blk = [None, None]
            xkv_blk = [None] * NKB

            xq_blk[0] = stream.tile([128, FC, 512], f16, tag="xblk",
                                    name="xqb0")
            xkv_blk[0] = stream.tile([128, FC, 512], f16, tag="xblk",
                                     name="xkvb0")
            for fc in range(FC):
                nc.sync.dma_start(out=wq_sb[:, fc, :], in_=wq_r[:, fc, :])
                nc.sync.dma_start(out=xq_blk[0][:, fc, :],
                                  in_=xqT_r[:, fc, 0:512])
            for fc in range(FC):
                nc.scalar.dma_start(out=wk_sb[:, fc, :], in_=wk_r[:, fc, :])
                nc.scalar.dma_start(out=xkv_blk[0][:, fc, :],
                                  in_=xkvT_r[:, fc, 0:512])
            nc.scalar.dma_start(out=wv_sb[:], in_=wv_r[:])
            xq_blk[1] = stream.tile([128, FC, 512], f16, tag="xblk",
                                    name="xqb1")
            nc.sync.dma_start(out=xq_blk[1][:], in_=xqT_r[:, :, 512:1024])
            nc.sync.dma_start(out=wo_bf[:], in_=wo_r[:])
            nc.sync.dma_start(out=ones_sb[64:65, :], in_=ones64[:])

            def emit_qproj(qb, hdcs=range(HDC)):
                for hdc in hdcs:
                    acc = psum.tile([128, 512], f32, tag="proj",
                                    name=f"qacc{qb}_{hdc}")
                    for fc in range(FC):
                        nc.tensor.matmul(
                            acc[:],
                            wq_sb[:, fc, hdc * 128:(hdc + 1) * 128],
                            xq_blk[qb][:, fc, :],
                            start=(fc == 0), stop=(fc == FC - 1),
                        )
                    nc.vector.tensor_copy(
                        qT_sb[:, hdc, qb * 512:(qb + 1) * 512], acc[:])

            def emit_kproj(kb, hdcs=range(HDC)):
                blk = xkv_blk[kb]
                for hdc in hdcs:
                    acc = psum.tile([128, 512], f32, tag="proj",
                                    name=f"kacc{kb}_{hdc}")
                    for fc in range(FC):
                        nc.tensor.matmul(
                            acc[:],
                            wk_sb[:, fc, hdc * 128:(hdc + 1) * 128],
                            blk[:, fc, :],
                            start=(fc == 0), stop=(fc == FC - 1),
                        )
                    nc.vector.tensor_copy(kT_sb[kb][:, hdc, :], acc[:])

            def emit_vproj(kb):
                blk = xkv_blk[kb]
                for ks in range(4):
                    kc = kb * 4 + ks
                    acc = psum.tile([128, 512], f32, tag="proj",
                                    name=f"vacc{kb}_{ks}")
                    for fc in range(FC):
                        nc.tensor.matmul(
                            acc[:],
                            blk[:, fc, ks * 128:(ks + 1) * 128],
                            wv_sb[:, fc, :],
                            start=(fc == 0), stop=(fc == FC - 1),
                        )
                    nc.vector.tensor_copy(
                        v_sb[kc][:, :, 0:64],
                        acc.rearrange("p (h d) -> p h d", h=H))
                    nc.vector.memset(v_sb[kc][:, :, 64:65], 1.0)

            # ---- attention emission helpers ----
            def emit_st_exp(hp, qb, kc, cached=False):
                kb, ks = kc // 4, kc % 4
                st = psum.tile([128, 2, 512], f32, tag="st",
                               name=f"st{hp}_{qb}_{kc}")
                for hi in range(2):
                    nc.tensor.matmul(
                        st[:, hi, :],
                        kT_sb[kb][hi * 64:(hi + 1) * 64, hp,
                                  ks * 128:(ks + 1) * 128],
                        qT_sb[hi * 64:(hi + 1) * 64, hp,
                              qb * 512:(qb + 1) * 512],
                        start=True, stop=True,
                        tile_position=(hi * 64, 0),
                    )
                pT = ptp.tile([128, 2, 512], bf16,
                              tag="pTc" if cached else "pT",
                              bufs=28 if cached else 12,
                              name=f"pT{hp}_{qb}_{kc}")
                nc.scalar.activation(
                    out=pT[:], in_=st[:],
                    func=mybir.ActivationFunctionType.Exp)
                return pT

            def emit_av(hp, qb, kc, av, pT):
                for hi in range(2):
                    nc.tensor.matmul(
                        av[hi][:],
                        v_sb[kc][:, hp * 2 + hi, :],
                        pT[:, hi, :],
                        start=(kc == 0), stop=(kc == NKC - 1),
                    )

            def alloc_av(hp, qb):
                return [psum.tile([65, 512], f32, tag="av",
                                  name=f"av{hp}_{qb}_{hi}")
                        for hi in range(2)]

            def emit_u_copies(hp, qb, av):
                us = []
                for hi in range(2):
                    u = small.tile([65, 512], f32r, tag="uav", bufs=3,
                                   name=f"u{hp}_{qb}_{hi}")
                    with nc.allow_low_precision(
                            reason="f32r attn output staging"):
                        nc.vector.tensor_copy(u[:], av[hi][:])
                    us.append(u)
                return (hp, qb, us)

            def emit_outproj(qb):
                # out[q, F] = sum_hd outT[hd, q]^T x wo[hd, F] for this qb
                for j in range(4):
                    qc = qb * 4 + j
                    acc = psum.tile([128, 512], f32, tag="proj",
                                    name=f"oacc{qc}")
                    for hdc in range(HDC):
                        nc.tensor.matmul(
                            acc[:],
                            outT_sb[:, hdc, qc * 128:(qc + 1) * 128],
                            wo_bf[:, hdc, :],
                            start=(hdc == 0), stop=(hdc == HDC - 1),
                        )
                    ostage = small.tile([128, 512], f32, tag="ost", bufs=2,
                                        name=f"ost{qc}")
                    nc.vector.tensor_copy(ostage[:], acc[:])
                    nc.sync.dma_start(out=out[qc * 128:(qc + 1) * 128, :],
                                      in_=ostage[:])

            def bcast_den(u, name):
                # broadcast u's denominator row across 64 partitions via a
                # tiny ones^T matmul (PE waits only on the fast av->u copy)
                bc = psum.tile([64, 512], f32, tag="proj", name=name)
                nc.tensor.matmul(bc[:], ones_sb[64:65, :], u[64:65, :],
                                 start=True, stop=True)
                return bc

            def emit_tail_rest(p):
                hp, qb, us = p
                bcs = [bcast_den(us[hi], f"bc{hp}_{qb}_{hi}")
                       for hi in range(2)]
                for hi in range(2):
                    rbc = small.tile([64, 512], f32, tag="rbc", bufs=2,
                                     name=f"rbc{hp}_{qb}_{hi}")
                    nc.vector.reciprocal(rbc[:], bcs[hi][:])
                    nc.vector.tensor_mul(
                        outT_sb[hi * 64:(hi + 1) * 64, hp,
                                qb * 512:(qb + 1) * 512],
                        us[hi][0:64, :], rbc[:])

            # ---- phase 2: two-cursor S^T/exp vs AV emission ----
            # The S^T+exp (SE) stream runs up to CAP slabs ahead of the AV
            # stream, decoupled through the pT pool. During the ramp the SE
            # order is kb-major across the first two iterations so the
            # Scalar engine is fed ~8 exps per projected k-block; afterwards
            # it is iteration-major. The AV stream is iteration-major
            # (PSUM accumulator lifetime), with the normalization tails and
            # per-qb output projections woven in at fixed offsets.
            ROLL = 12          # rolling slab pool depth (tag pT)
            iters = [(hp, qb) for hp in range(NHP) for qb in range(NQB)]
            NIT = len(iters)

            # SE order: ramp kb-major — iteration 0 always, iteration 1
            # cached (long-lived slabs) for kb>=4 — then iteration 1's
            # remaining k-chunks, then iterations 2.. in order.
            se_list = []   # (iter_idx, kc, cached)
            for kb in range(NKB):
                se_list.extend((0, kb * 4 + ks, False) for ks in range(4))
                if 2 <= kb <= 6:
                    se_list.extend((1, kb * 4 + ks, True) for ks in range(4))
                if kb == 2:
                    # iteration 1's first k-blocks, emitted once qT(qb1) exists
                    se_list.extend((1, ks, True) for ks in range(8))
            se_list.extend((1, kc, False) for kc in range(28, 32))
            for i in range(2, NIT):
                se_list.extend((i, kc, False) for kc in range(NKC))
            av_list = [(i, kc) for i in range(NIT) for kc in range(NKC)]
            av_idx = {step: j for j, step in enumerate(av_list)}
            # rolling-tag allocation order, for the FIFO slot-reuse guard
            roll_steps = [s for s in se_list if not s[2]]

            cache = {}
            state = dict(se_pos=0, roll_pos=0, av_pos=0, pending=None,
                         outproj_due=None, av_tiles=None, kb_done=0)

            def pump_se(budget):
                n = 0
                while n < budget and state["se_pos"] < len(se_list):
                    i, kc, cached = se_list[state["se_pos"]]
                    if kc // 4 >= state["kb_done"]:
                        break  # K^T/V for this k-block not yet emitted
                    if not cached:
                        r = state["roll_pos"]
                        if r >= ROLL:
                            blocker = roll_steps[r - ROLL]
                            if av_idx[blocker[:2]] >= state["av_pos"]:
                                break  # rolling slot's consumer not emitted
                        state["roll_pos"] = r + 1
                    hp, qb = iters[i]
                    cache[(i, kc)] = emit_st_exp(hp, qb, kc, cached)
                    state["se_pos"] += 1
                    n += 1

            def emit_av_step():
                i, kc = av_list[state["av_pos"]]
                hp, qb = iters[i]
                if kc == 0:
                    state["av_tiles"] = alloc_av(hp, qb)
                emit_av(hp, qb, kc, state["av_tiles"], cache.pop((i, kc)))
                if kc == 4 and state["pending"] is not None:
                    emit_tail_rest(state["pending"])
                    state["outproj_due"] = (state["pending"][1]
                                            if state["pending"][0] == NHP - 1
                                            else None)
                    state["pending"] = None
                if kc == 16 and state["outproj_due"] is not None:
                    emit_outproj(state["outproj_due"])
                    state["outproj_due"] = None
                if kc == NKC - 1 and i < NIT - 1:
                    state["pending"] = emit_u_copies(hp, qb, state["av_tiles"])
                state["av_pos"] += 1

            def av_ready():
                # next AV step's slab is already emitted
                return (state["av_pos"] < len(av_list)
                        and av_list[state["av_pos"]] in cache)

            emit_qproj(0, hdcs=[0])
            for kb in range(NKB):
                if kb > 0:
                    xkv_blk[kb] = stream.tile([128, FC, 512], f16,
                                              tag="xblk", name=f"xkvb{kb}")
                    nc.sync.dma_start(
                        out=xkv_blk[kb][:],
                        in_=xkvT_r[:, :, kb * 512:(kb + 1) * 512])
                # hdc0 of K^T unblocks all head-pair-0 S^T for this k-block
                emit_kproj(kb, hdcs=[0])
                state["kb_done"] = kb + 1
                pump_se(4)
                emit_kproj(kb, hdcs=[1, 2, 3])
                if kb == 0:
                    emit_qproj(0, hdcs=[1, 2, 3])
                pump_se(2)
                emit_vproj(kb)
                if kb == 0:
                    emit_qproj(1)
                pump_se(8)
                while av_ready():
                    emit_av_step()
                    pump_se(1)

            while state["av_pos"] < len(av_list):
                pump_se(2)
                if not av_ready():
                    pump_se(len(se_list))
                emit_av_step()

            # final tail, pipelined per 128-wide q chunk so the output
            # projection starts as soon as each slice is normalized
            fi = NIT - 1
            fhp, fqb = iters[fi]
            fus = emit_u_copies(fhp, fqb, state["av_tiles"])[2]
            fbcs = [bcast_den(fus[hi], f"fbc{hi}") for hi in range(2)]
            for j in range(4):
                qc = fqb * 4 + j
                js = slice(j * 128, (j + 1) * 128)
                for hi in range(2):
                    rbc = small.tile([64, 128], f32, tag="frbc",
                                     name=f"frbc{j}_{hi}")
                    nc.vector.reciprocal(rbc[:], fbcs[hi][:, js])
                    nc.vector.tensor_mul(
                        outT_sb[hi * 64:(hi + 1) * 64, fhp,
                                fqb * 512 + j * 128:fqb * 512 + (j + 1) * 128],
                        fus[hi][0:64, js], rbc[:])
                acc = psum.tile([128, 512], f32, tag="st", name=f"foacc{qc}")
                for hdc in range(HDC):
                    nc.tensor.matmul(
                        acc[:],
                        outT_sb[:, hdc, qc * 128:(qc + 1) * 128],
                        wo_bf[:, hdc, :],
                        start=(hdc == 0), stop=(hdc == HDC - 1),
                    )
                ostage = small.tile([128, 512], f32, tag="ost", bufs=2,
                                    name=f"fost{qc}")
                nc.vector.tensor_copy(ostage[:], acc[:])
                nc.sync.dma_start(out=out[qc * 128:(qc + 1) * 128, :],
                                  in_=ostage[:])

    nc.compile()
    return nc


def kernel(**inputs):
    global last_result
    import ml_dtypes
    inputs_q = np.asarray(inputs["inputs_q"], dtype=np.float32)
    inputs_kv = np.asarray(inputs["inputs_kv"], dtype=np.float32)
    Wq = np.asarray(inputs["Wq"], dtype=np.float32).reshape(F, HD).astype(np.float16)
    Wk = np.asarray(inputs["Wk"], dtype=np.float32).reshape(F, HD).astype(np.float16)
    Wv = np.asarray(inputs["Wv"], dtype=np.float32).reshape(F, HD).astype(np.float16)
    Wo = np.asarray(inputs["Wo"], dtype=np.float32).reshape(HD, F).astype(ml_dtypes.bfloat16)
    ones = np.ones((1, 64), dtype=np.float32)

    if "nc" not in _cache:
        _cache["nc"] = _build_program()
    nc = _cache["nc"]

    xkvT = [np.ascontiguousarray(inputs_kv[b].T).astype(np.float16) for b in range(B)]
    in_maps = []
    for c in range(NCORES):
        b, qi = c // 4, c % 4
        in_maps.append({
            "xqT": np.ascontiguousarray(
                inputs_q[b, qi * QSH:(qi + 1) * QSH, :].T).astype(np.float16),
            "xkvT": xkvT[b],
            "wq": Wq, "wk": Wk, "wv": Wv, "wo": Wo,
            "ones64": ones,
        })

    res = run_bass_kernel_spmd(nc, in_maps, core_ids=list(range(NCORES)))
    last_result = res

    out = np.empty((B, Q, F), dtype=np.float32)
    for c in range(NCORES):
        b, qi = c // 4, c % 4
        out[b, qi * QSH:(qi + 1) * QSH, :] = res.results[c]["out"]
    return out
